# revision 1
# baseline (speedup 1.0000x reference)
"""BoundaryMaxPooling Trainium2 kernel, v2: bf16 pair-packed sparse tables.

Per core (data-parallel over B=8 batches): channel tiles are processed in
PAIRS (A, B).  A is cast to bf16 into the even 16-bit halves and B into the
odd halves of an int32 "packed" tile.  The sparse-table doubling max then
runs on the bf16 *view* of the packed tile — DVE's 2x_1P mode processes 2
bf16/cycle, so one pass builds both tiles' tables in the cycles one f32
table used to take.  Each RMQ lookup is ONE int32 ap_gather that fetches
both tiles' values at once (halving GPSIMD work).  Casts (f32->bf16
interleave) and unpacks (bf16 deinterleave -> f32) run on the otherwise
idle Activation engine.

Levels are stored compactly: level k holds T+1-2^k entries at offset
off[k] = sum_{j<k} (T+1-2^j); host-computed indices bake these offsets in.
"""

import numpy as np

B, C2, T = 8, 1024, 2048
KLEV = 9
P = 128
N_CORES = 8
N_TILES = C2 // P          # 8 channel tiles per batch
N_PAIRS = N_TILES // 2     # 4 pairs per batch

LEV_N = [T + 1 - (1 << k) for k in range(KLEV)]       # entries per level
LEV_OFF = [0]
for k in range(1, KLEV):
    LEV_OFF.append(LEV_OFF[-1] + LEV_N[k - 1])
NE_C = LEV_OFF[-1] + LEV_N[-1]                         # 17930 int32 elems

# All build levels run on DVE.  (GPSIMD cannot take tensor_tensor work:
# its Q7 ucode library is selected per-kernel, and ap_gather + tensor_tensor
# live in different libraries — the Pool ISA check rejects the combination.)
DVE_FRAC = 1.0

_CACHE = {}


def _build_program():
    import concourse.bacc as bacc
    import concourse.mybir as mybir
    import concourse.tile as tile

    f32 = mybir.dt.float32
    bf16 = mybir.dt.bfloat16
    i16 = mybir.dt.int16
    i32 = mybir.dt.int32
    MAX = mybir.AluOpType.max
    COPY = mybir.ActivationFunctionType.Copy

    nc = bacc.Bacc("TRN2", target_bir_lowering=False, debug=False,
                   num_devices=N_CORES)
    feat = nc.dram_tensor("feat", [C2, T], f32, kind="ExternalInput")
    idxw = nc.dram_tensor("idxw", [P, 512], i16, kind="ExternalInput")
    out = nc.dram_tensor("out", [C2, T], f32, kind="ExternalOutput")

    with tile.TileContext(nc) as tc:
        with tc.tile_pool(name="idxp", bufs=1) as ip, \
             tc.tile_pool(name="inp", bufs=1) as inp, \
             tc.tile_pool(name="tabp", bufs=2) as tp, \
             tc.tile_pool(name="gp", bufs=2) as gp, \
             tc.tile_pool(name="op", bufs=1) as op_:
            idxt = ip.tile([P, 512], i16, tag="idx")
            nc.sync.dma_start(idxt[:], idxw[:])
            for j in range(N_PAIRS):
                tA, tB = 2 * j, 2 * j + 1
                half = tA // (N_TILES // 2)  # 0 = start half, 1 = end half
                inA = inp.tile([P, T], f32, tag="inA")
                inB = inp.tile([P, T], f32, tag="inB")
                nc.sync.dma_start(inA[:], feat[tA * P:(tA + 1) * P, :])
                nc.sync.dma_start(inB[:], feat[tB * P:(tB + 1) * P, :])

                tab = tp.tile([P, NE_C], i32, tag="tab")
                tb = tab[:].bitcast(bf16)  # [P, 2*NE_C]
                # level 0: cast+interleave A -> even halves, B -> odd halves
                nc.scalar.activation(tb[:, 0:2 * T:2], inA[:], COPY)
                nc.scalar.activation(tb[:, 1:2 * T:2], inB[:], COPY)
                # levels 1..8: doubling max on the bf16 view (2 bf16/elem)
                for k in range(1, KLEV):
                    s = 1 << (k - 1)
                    n = LEV_N[k]
                    o, po = LEV_OFF[k], LEV_OFF[k - 1]
                    nd = n if DVE_FRAC >= 1.0 else (int(n * DVE_FRAC) // 2) * 2
                    nc.vector.tensor_tensor(
                        tb[:, 2 * o:2 * (o + nd)],
                        tb[:, 2 * po:2 * (po + nd)],
                        tb[:, 2 * (po + s):2 * (po + s + nd)],
                        MAX)
                    if n > nd:
                        nc.gpsimd.tensor_tensor(
                            tb[:, 2 * (o + nd):2 * (o + n)],
                            tb[:, 2 * (po + nd):2 * (po + n)],
                            tb[:, 2 * (po + s + nd):2 * (po + s + n)],
                            MAX)
                g1 = gp.tile([P, T], i32, tag="g1")
                g2 = gp.tile([P, T], i32, tag="g2")
                colA = half * 256
                colB = colA + 128
                nc.gpsimd.ap_gather(
                    g1[:], tab[:], idxt[:, colA:colA + 128],
                    channels=P, num_elems=NE_C, d=1, num_idxs=T)
                nc.gpsimd.ap_gather(
                    g2[:], tab[:], idxt[:, colB:colB + 128],
                    channels=P, num_elems=NE_C, d=1, num_idxs=T)
                # elementwise max of the two packed lookups (both tiles at once)
                nc.vector.tensor_tensor(
                    g1[:].bitcast(bf16), g1[:].bitcast(bf16),
                    g2[:].bitcast(bf16), MAX)
                # deinterleave + upcast to f32 on the Activation engine
                ob = op_.tile([P, 2 * T], f32, tag="ob")
                gb = g1[:].bitcast(bf16)
                nc.scalar.activation(ob[:, 0:T], gb[:, 0:2 * T:2], COPY)
                nc.scalar.activation(ob[:, T:2 * T], gb[:, 1:2 * T:2], COPY)
                nc.sync.dma_start(out[tA * P:(tA + 1) * P, :], ob[:, 0:T])
                nc.sync.dma_start(out[tB * P:(tB + 1) * P, :], ob[:, T:2 * T])
    nc.compile()
    return nc


def _host_indices(segments, max_len):
    """Window indices for batch-0 segments, in compact-level layout.

    Returns wrapped-int16 [128, 512] with 4 column groups:
    [A_start | B_start | A_end | B_end], each 128 cols of 2048 wrapped idx.
    """
    seg = np.asarray(segments, dtype=np.float32)
    seg0 = np.clip(seg[0], 0.0, np.float32(max_len - 1))  # [T, 4]
    off = np.asarray(LEV_OFF, dtype=np.int64)

    def win(lo_col, hi_col):
        lo = np.floor(seg0[:, lo_col]).astype(np.int64)
        hi = np.ceil(seg0[:, hi_col]).astype(np.int64)
        hi = np.maximum(hi, lo + 1)
        return lo, hi

    def level_idx(lo, hi):
        L = hi - lo
        k = np.floor(np.log2(L.astype(np.float64))).astype(np.int64)
        i1 = off[k] + lo
        i2 = off[k] + hi - (1 << k)
        return i1.astype(np.int16), i2.astype(np.int16)

    def wrap(idx):
        # element i -> partition i % 16, col i // 16, replicated per 16-group
        blk = np.asarray(idx).reshape(-1, 16).T  # [16, n/16]
        return np.tile(blk, (8, 1)).astype(np.int16)  # [128, n/16]

    lo_s, hi_s = win(0, 1)
    lo_e, hi_e = win(2, 3)
    a_s, b_s = level_idx(lo_s, hi_s)
    a_e, b_e = level_idx(lo_e, hi_e)
    return np.concatenate(
        [wrap(a_s), wrap(b_s), wrap(a_e), wrap(b_e)], axis=1)


def kernel(feature, segments, max_len=T, **_unused):
    from concourse import bass_utils

    feature = np.asarray(feature, dtype=np.float32)
    assert feature.shape == (B, C2, T), feature.shape
    idxw = _host_indices(segments, int(max_len))

    if "nc" not in _CACHE:
        _CACHE["nc"] = _build_program()
    nc = _CACHE["nc"]

    in_maps = [
        {"feat": np.ascontiguousarray(feature[b]), "idxw": idxw}
        for b in range(B)
    ]
    res = bass_utils.run_bass_kernel_spmd(
        nc, in_maps, core_ids=list(range(N_CORES)))
    return np.stack([res.results[b]["out"] for b in range(B)], axis=0)



# revision 9
# speedup vs baseline: 2.6348x; 2.6348x over previous
"""BoundaryMaxPooling Trainium2 kernel, v7: half-time d=8 packed tables.

ap_gather cost is per-index and nearly flat in d (d=8 is only ~14%/idx
over d=4 while fetching 2x the data).  v7 therefore splits the TIME axis
in half (with a 257-wide halo, the max segment length) so SIXTEEN
half-tiles share one index stream, packed as 16 bf16 lanes per 32-byte
entry (d=8).  Each query belongs to exactly one half (by its lo), so the
per-core index count halves vs v4-v6.

Sharding: 8 cores = 2 families x 2 time-halves x 2 batch-groups.
Core c: family f=c//4 (start/end segments), half h=(c//2)%2 (data window
[767h, 767h+1281)), batch group i=c%2 (batches 4i..4i+3).  16 lanes =
4 batches x 4 channel-blocks of that family.

All 9 levels live in ONE tile pool of 4 rotating 41-KB buffers:
L0->b0, L1->b1, L2->b2, L3->b3, L4->b0, L5->b1, L6->b2, L7->b3, L8->b0.
Level k+4 overwrites level k's buffer, which class-k gathers read; since
gather SOURCE reads carry no completion semantics, each of L5..L8 is
preceded by a 2-column copy of class-k's last gather OUTPUT into the
target buffer -- the WAW edge with the build serializes it correctly
(L4 needs no guard: level 0 is only read by the tracked L1 build; k=0
queries are a host passthrough).

Each chunk's index stream is [all probe-1 | all probe-2]; the pairwise
max runs in place on the gather output.  Host packs bf16 input,
deinterleaves/upcasts/un-permutes the output.
"""

import numpy as np

B, C2, T = 8, 1024, 2048
P = 128
N_CORES = 8
KLEV = 9
HT = 1281                                # half window: 1024 + 257 halo
H_OFF = [0, 767]                         # window start per half
HN = [HT + 1 - (1 << k) for k in range(KLEV)]

MAXCH = 256                              # max queries per gather chunk
GROUPS = [1, 2, 3, 4, 5, 6, 7, 8]

_CACHE = {}


def _plan_chunks(counts):
    chunks = []
    for g in GROUPS:
        n = counts[g]
        while n > 0:
            c = min(n, MAXCH)
            chunks.append((g, ((c + 15) // 16) * 16))
            n -= c
    return chunks


def _build_program(chunks):
    import concourse.bacc as bacc
    import concourse.mybir as mybir
    import concourse.tile as tile

    bf16 = mybir.dt.bfloat16
    i16 = mybir.dt.int16
    i32 = mybir.dt.int32
    MAX = mybir.AluOpType.max

    qtot = sum(c for _, c in chunks)
    idxcols = 2 * qtot // 16

    nc = bacc.Bacc("TRN2", target_bir_lowering=False, debug=False,
                   num_devices=N_CORES)
    feat = nc.dram_tensor("feat", [P, 8 * HT], i32, kind="ExternalInput")
    idxw = nc.dram_tensor("idxw", [P, idxcols], i16, kind="ExternalInput")
    out = nc.dram_tensor("out", [P, 8 * qtot], i32, kind="ExternalOutput")

    with tile.TileContext(nc) as tc:
        with tc.tile_pool(name="tabp", bufs=4) as tp, \
             tc.tile_pool(name="gp", bufs=2) as gp, \
             tc.tile_pool(name="ip", bufs=1) as ip:
            idxt = ip.tile([P, idxcols], i16, tag="idx")
            nc.sync.dma_start(idxt[:], idxw[:])

            lev = [None] * KLEV
            lev[0] = tp.tile([P, 8 * HT], i32, tag="t", name="lev0")
            nc.sync.dma_start(lev[0][:, 0:4 * HT], feat[:, 0:4 * HT])
            nc.sync.dma_start(lev[0][:, 4 * HT:8 * HT], feat[:, 4 * HT:])

            state = {"col": 0, "ocol": 0, "last": {}}

            def do_chunks(k):
                src = lev[k]
                for g, ch in chunks:
                    if g != k:
                        continue
                    ni = 2 * ch
                    gt = gp.tile([P, 16 * MAXCH], i32, tag="g")
                    nc.gpsimd.ap_gather(
                        gt[:, 0:8 * ni], src[:, 0:8 * HN[k]],
                        idxt[:, state["col"]:state["col"] + ni // 16],
                        channels=P, num_elems=HN[k], d=8, num_idxs=ni)
                    state["last"][k] = gt
                    gb = gt[:].bitcast(bf16)
                    nc.vector.tensor_tensor(
                        gb[:, 0:16 * ch],
                        gb[:, 0:16 * ch], gb[:, 16 * ch:32 * ch], MAX)
                    nc.sync.dma_start(
                        out[:, state["ocol"]:state["ocol"] + 8 * ch],
                        gt[:, 0:8 * ch])
                    state["col"] += ni // 16
                    state["ocol"] += 8 * ch

            for k in range(1, KLEV):
                lev[k] = tp.tile([P, 8 * HT], i32, tag="t",
                                 name=f"lev{k}")
                if k >= 5:
                    # guard: class k-4 gathers read the buffer this level
                    # overwrites; chain through their gather OUTPUT
                    ga = state["last"].get(k - 4)
                    if ga is not None:
                        nc.vector.tensor_copy(lev[k][:, 0:2], ga[:, 0:2])
                s = 1 << (k - 1)
                vo = lev[k][:].bitcast(bf16)
                vi = lev[k - 1][:].bitcast(bf16)
                nc.vector.tensor_tensor(
                    vo[:, 0:16 * HN[k]],
                    vi[:, 0:16 * HN[k]],
                    vi[:, 16 * s:16 * (s + HN[k])], MAX)
                do_chunks(k)
    nc.compile()
    return nc


def _f32_to_bf16_u16(x):
    u = x.astype(np.float32).view(np.uint32)
    rounded = u + 0x7FFF + ((u >> 16) & 1)
    return (rounded >> 16).astype(np.uint16)


def _queries(segments, max_len):
    seg = np.clip(np.asarray(segments, np.float32)[0], 0.0,
                  np.float32(max_len - 1))
    fams = []
    for f in (0, 1):
        lo = np.floor(seg[:, 2 * f]).astype(np.int64)
        hi = np.ceil(seg[:, 2 * f + 1]).astype(np.int64)
        hi = np.maximum(hi, lo + 1)
        ln = hi - lo
        k = np.floor(np.log2(ln.astype(np.float64))).astype(np.int64)
        fams.append((k, lo, hi - (1 << k)))
    return fams


def _layout(fams):
    """Chunk layout unified across the four (family, half) streams."""
    streams = []
    for f in (0, 1):
        k, p1, p2 = fams[f]
        for h in (0, 1):
            m = (k >= 1) & ((p1 >= 1024) == bool(h))
            streams.append((f, h, k, p1, p2, m))
    counts = {g: 0 for g in GROUPS}
    for _, _, k, _, _, m in streams:
        for g in GROUPS:
            counts[g] = max(counts[g], int(np.sum(m & (k == g))))
    chunks = _plan_chunks(counts)
    qtot = sum(c for _, c in chunks)

    lay = {}
    for f, h, k, p1, p2, m in streams:
        tsort = {g: np.nonzero(m & (k == g))[0] for g in GROUPS}
        used = {g: 0 for g in GROUPS}
        idx_stream, perm = [], []
        for g, ch in chunks:
            ts = tsort[g][used[g]:used[g] + ch]
            used[g] += ch
            npad = ch - len(ts)
            a = np.concatenate([p1[ts] - H_OFF[h],
                                np.zeros(npad, np.int64)])
            b = np.concatenate([p2[ts] - H_OFF[h],
                                np.zeros(npad, np.int64)])
            assert (a >= 0).all() and (b >= 0).all()
            assert (a < HN[g]).all() if len(ts) else True
            idx_stream.append(np.concatenate([a, b]))
            perm.append(np.concatenate([ts, -np.ones(npad, np.int64)]))
        idx = np.concatenate(idx_stream).astype(np.int16)
        assert idx.size == 2 * qtot
        wrapped = np.tile(idx.reshape(-1, 16).T, (8, 1)).astype(np.int16)
        lay[(f, h)] = (wrapped, np.concatenate(perm))
    k0s = []
    for f in (0, 1):
        k, p1, _ = fams[f]
        t0 = np.nonzero(k == 0)[0]
        k0s.append((t0, p1[t0]))
    return chunks, lay, k0s


def prepare(feature, segments, max_len):
    feature = np.asarray(feature, np.float32)
    u16 = _f32_to_bf16_u16(feature)           # [B, C2, T]
    fams = _queries(segments, int(max_len))
    chunks, lay, k0s = _layout(fams)
    in_maps, perms = [], []
    for c in range(N_CORES):
        f, h, i = c // 4, (c // 2) % 2, c % 2
        # lanes j = 0..15: batch 4i + j//4, channels 512f + 128*(j%4) + p
        x = u16[4 * i:4 * i + 4, 512 * f:512 * (f + 1),
                H_OFF[h]:H_OFF[h] + HT]                    # [4,512,HT]
        x = x.reshape(4, 4, P, HT).transpose(2, 3, 0, 1)   # [p,e,b,cb]
        packed = np.ascontiguousarray(x.reshape(P, HT, 16)).view(np.uint32)
        packed = packed.reshape(P, 8 * HT).astype(np.int32, copy=False)
        wrapped, perm = lay[(f, h)]
        in_maps.append({"feat": packed, "idxw": wrapped})
        perms.append(perm)
    return chunks, in_maps, perms, k0s


def postprocess(results, perms, k0s, feature):
    feature = np.asarray(feature, np.float32)
    out = np.empty((B, C2, T), np.float32)
    for c in range(N_CORES):
        f, h, i = c // 4, (c // 2) % 2, c % 2
        r = np.asarray(results[c]["out"])          # [P, 8*qtot] i32
        qtot = r.shape[1] // 8
        u16 = r.view(np.uint16).reshape(P, qtot, 16)
        perm = perms[c]
        valid = perm >= 0
        tq = perm[valid]
        v = u16[:, valid, :]                       # [P, nq, 16]
        f32 = (v.astype(np.uint32) << 16).view(np.float32)
        f32 = f32.transpose(2, 0, 1).reshape(4, 4, P, -1).reshape(4, 512, -1)
        out[4 * i:4 * i + 4, 512 * f:512 * (f + 1), :][:, :, tq] = f32
    for f in (0, 1):
        t0, lo0 = k0s[f]
        if len(t0):
            out[:, 512 * f:512 * (f + 1), t0] = \
                feature[:, 512 * f:512 * (f + 1), lo0]
    return out


def kernel(feature, segments, max_len=T, **_unused):
    from concourse import bass_utils

    feature = np.asarray(feature, dtype=np.float32)
    assert feature.shape == (B, C2, T), feature.shape
    chunks, in_maps, perms, k0s = prepare(feature, segments, int(max_len))

    key = tuple(chunks)
    if _CACHE.get("key") != key:
        _CACHE["nc"] = _build_program(chunks)
        _CACHE["key"] = key
    nc = _CACHE["nc"]

    res = bass_utils.run_bass_kernel_spmd(
        nc, in_maps, core_ids=list(range(N_CORES)))
    return postprocess(res.results, perms, k0s, feature)


# revision 10
# speedup vs baseline: 2.9447x; 1.1176x over previous
"""BoundaryMaxPooling Trainium2 kernel, v7: half-time d=8 packed tables.

ap_gather cost is per-index and nearly flat in d (d=8 is only ~14%/idx
over d=4 while fetching 2x the data).  v7 therefore splits the TIME axis
in half (with a 257-wide halo, the max segment length) so SIXTEEN
half-tiles share one index stream, packed as 16 bf16 lanes per 32-byte
entry (d=8).  Each query belongs to exactly one half (by its lo), so the
per-core index count halves vs v4-v6.

Sharding: 8 cores = 2 families x 2 time-halves x 2 batch-groups.
Core c: family f=c//4 (start/end segments), half h=(c//2)%2 (data window
[767h, 767h+1281)), batch group i=c%2 (batches 4i..4i+3).  16 lanes =
4 batches x 4 channel-blocks of that family.

All 9 levels live in ONE tile pool of 4 rotating 41-KB buffers:
L0->b0, L1->b1, L2->b2, L3->b3, L4->b0, L5->b1, L6->b2, L7->b3, L8->b0.
Level k+4 overwrites level k's buffer, which class-k gathers read; since
gather SOURCE reads carry no completion semantics, each of L5..L8 is
preceded by a 2-column copy of class-k's last gather OUTPUT into the
target buffer -- the WAW edge with the build serializes it correctly
(L4 needs no guard: level 0 is only read by the tracked L1 build; k=0
queries are a host passthrough).

Each chunk's index stream is [all probe-1 | all probe-2]; the pairwise
max runs in place on the gather output.  Host packs bf16 input,
deinterleaves/upcasts/un-permutes the output.
"""

import numpy as np

B, C2, T = 8, 1024, 2048
P = 128
N_CORES = 8
KLEV = 9
HT = 1281                                # half window: 1024 + 257 halo
H_OFF = [0, 767]                         # window start per half
HN = [HT + 1 - (1 << k) for k in range(KLEV)]

MAXCH = 320                              # max queries per gather chunk
GROUPS = [1, 2, 3, 4, 5, 6, 7, 8]

_CACHE = {}


def _plan_chunks(counts):
    chunks = []
    for g in GROUPS:
        n = counts[g]
        while n > 0:
            c = min(n, MAXCH)
            chunks.append((g, ((c + 15) // 16) * 16))
            n -= c
    return chunks


def _build_program(chunks):
    import concourse.bacc as bacc
    import concourse.mybir as mybir
    import concourse.tile as tile

    bf16 = mybir.dt.bfloat16
    i16 = mybir.dt.int16
    i32 = mybir.dt.int32
    MAX = mybir.AluOpType.max

    qtot = sum(c for _, c in chunks)
    idxcols = 2 * qtot // 16

    nc = bacc.Bacc("TRN2", target_bir_lowering=False, debug=False,
                   num_devices=N_CORES)
    feat = nc.dram_tensor("feat", [P, 8 * HT], i32, kind="ExternalInput")
    idxw = nc.dram_tensor("idxw", [P, idxcols], i16, kind="ExternalInput")
    out = nc.dram_tensor("out", [P, 8 * qtot], i32, kind="ExternalOutput")

    with tile.TileContext(nc) as tc:
        with tc.tile_pool(name="tabp", bufs=4) as tp, \
             tc.tile_pool(name="gp", bufs=2) as gp, \
             tc.tile_pool(name="ip", bufs=1) as ip:
            idxt = ip.tile([P, idxcols], i16, tag="idx")
            nc.sync.dma_start(idxt[:], idxw[:])

            lev = [None] * KLEV
            lev[0] = tp.tile([P, 8 * HT], i32, tag="t", name="lev0")
            nc.sync.dma_start(lev[0][:, 0:4 * HT], feat[:, 0:4 * HT])
            nc.sync.dma_start(lev[0][:, 4 * HT:8 * HT], feat[:, 4 * HT:])

            state = {"col": 0, "ocol": 0, "last": {}}

            def do_chunks(k):
                src = lev[k]
                for g, ch in chunks:
                    if g != k:
                        continue
                    ni = 2 * ch
                    gt = gp.tile([P, 16 * MAXCH], i32, tag="g")
                    nc.gpsimd.ap_gather(
                        gt[:, 0:8 * ni], src[:, 0:8 * HN[k]],
                        idxt[:, state["col"]:state["col"] + ni // 16],
                        channels=P, num_elems=HN[k], d=8, num_idxs=ni)
                    state["last"][k] = gt
                    gb = gt[:].bitcast(bf16)
                    nc.vector.tensor_tensor(
                        gb[:, 0:16 * ch],
                        gb[:, 0:16 * ch], gb[:, 16 * ch:32 * ch], MAX)
                    nc.sync.dma_start(
                        out[:, state["ocol"]:state["ocol"] + 8 * ch],
                        gt[:, 0:8 * ch])
                    state["col"] += ni // 16
                    state["ocol"] += 8 * ch

            for k in range(1, KLEV):
                lev[k] = tp.tile([P, 8 * HT], i32, tag="t",
                                 name=f"lev{k}")
                if k >= 5:
                    # guard: class k-4 gathers read the buffer this level
                    # overwrites; chain through their gather OUTPUT
                    ga = state["last"].get(k - 4)
                    if ga is not None:
                        nc.vector.tensor_copy(lev[k][:, 0:2], ga[:, 0:2])
                s = 1 << (k - 1)
                vo = lev[k][:].bitcast(bf16)
                vi = lev[k - 1][:].bitcast(bf16)
                if k == 1:
                    # two halves, pipelined with the split input DMA
                    half = HT // 2 - 2   # stay within the first DMA chunk
                    nc.vector.tensor_tensor(
                        vo[:, 0:16 * half],
                        vi[:, 0:16 * half],
                        vi[:, 16 * s:16 * (s + half)], MAX)
                    nc.vector.tensor_tensor(
                        vo[:, 16 * half:16 * HN[1]],
                        vi[:, 16 * half:16 * HN[1]],
                        vi[:, 16 * (half + s):16 * (s + HN[1])], MAX)
                else:
                    nc.vector.tensor_tensor(
                        vo[:, 0:16 * HN[k]],
                        vi[:, 0:16 * HN[k]],
                        vi[:, 16 * s:16 * (s + HN[k])], MAX)
                do_chunks(k)
    nc.compile()
    return nc


def _f32_to_bf16_u16(x):
    u = x.astype(np.float32).view(np.uint32)
    rounded = u + 0x7FFF + ((u >> 16) & 1)
    return (rounded >> 16).astype(np.uint16)


def _queries(segments, max_len):
    seg = np.clip(np.asarray(segments, np.float32)[0], 0.0,
                  np.float32(max_len - 1))
    fams = []
    for f in (0, 1):
        lo = np.floor(seg[:, 2 * f]).astype(np.int64)
        hi = np.ceil(seg[:, 2 * f + 1]).astype(np.int64)
        hi = np.maximum(hi, lo + 1)
        ln = hi - lo
        k = np.floor(np.log2(ln.astype(np.float64))).astype(np.int64)
        fams.append((k, lo, hi - (1 << k)))
    return fams


def _layout(fams):
    """Chunk layout unified across the four (family, half) streams."""
    streams = []
    for f in (0, 1):
        k, p1, p2 = fams[f]
        for h in (0, 1):
            m = (k >= 1) & ((p1 >= 1024) == bool(h))
            streams.append((f, h, k, p1, p2, m))
    counts = {g: 0 for g in GROUPS}
    for _, _, k, _, _, m in streams:
        for g in GROUPS:
            counts[g] = max(counts[g], int(np.sum(m & (k == g))))
    chunks = _plan_chunks(counts)
    qtot = sum(c for _, c in chunks)

    lay = {}
    for f, h, k, p1, p2, m in streams:
        tsort = {g: np.nonzero(m & (k == g))[0] for g in GROUPS}
        used = {g: 0 for g in GROUPS}
        idx_stream, perm = [], []
        for g, ch in chunks:
            ts = tsort[g][used[g]:used[g] + ch]
            used[g] += ch
            npad = ch - len(ts)
            a = np.concatenate([p1[ts] - H_OFF[h],
                                np.zeros(npad, np.int64)])
            b = np.concatenate([p2[ts] - H_OFF[h],
                                np.zeros(npad, np.int64)])
            assert (a >= 0).all() and (b >= 0).all()
            assert (a < HN[g]).all() if len(ts) else True
            idx_stream.append(np.concatenate([a, b]))
            perm.append(np.concatenate([ts, -np.ones(npad, np.int64)]))
        idx = np.concatenate(idx_stream).astype(np.int16)
        assert idx.size == 2 * qtot
        wrapped = np.tile(idx.reshape(-1, 16).T, (8, 1)).astype(np.int16)
        lay[(f, h)] = (wrapped, np.concatenate(perm))
    k0s = []
    for f in (0, 1):
        k, p1, _ = fams[f]
        t0 = np.nonzero(k == 0)[0]
        k0s.append((t0, p1[t0]))
    return chunks, lay, k0s


def prepare(feature, segments, max_len):
    feature = np.asarray(feature, np.float32)
    u16 = _f32_to_bf16_u16(feature)           # [B, C2, T]
    fams = _queries(segments, int(max_len))
    chunks, lay, k0s = _layout(fams)
    in_maps, perms = [], []
    for c in range(N_CORES):
        f, h, i = c // 4, (c // 2) % 2, c % 2
        # lanes j = 0..15: batch 4i + j//4, channels 512f + 128*(j%4) + p
        x = u16[4 * i:4 * i + 4, 512 * f:512 * (f + 1),
                H_OFF[h]:H_OFF[h] + HT]                    # [4,512,HT]
        x = x.reshape(4, 4, P, HT).transpose(2, 3, 0, 1)   # [p,e,b,cb]
        packed = np.ascontiguousarray(x.reshape(P, HT, 16)).view(np.uint32)
        packed = packed.reshape(P, 8 * HT).astype(np.int32, copy=False)
        wrapped, perm = lay[(f, h)]
        in_maps.append({"feat": packed, "idxw": wrapped})
        perms.append(perm)
    return chunks, in_maps, perms, k0s


def postprocess(results, perms, k0s, feature):
    feature = np.asarray(feature, np.float32)
    out = np.empty((B, C2, T), np.float32)
    for c in range(N_CORES):
        f, h, i = c // 4, (c // 2) % 2, c % 2
        r = np.asarray(results[c]["out"])          # [P, 8*qtot] i32
        qtot = r.shape[1] // 8
        u16 = r.view(np.uint16).reshape(P, qtot, 16)
        perm = perms[c]
        valid = perm >= 0
        tq = perm[valid]
        v = u16[:, valid, :]                       # [P, nq, 16]
        f32 = (v.astype(np.uint32) << 16).view(np.float32)
        f32 = f32.transpose(2, 0, 1).reshape(4, 4, P, -1).reshape(4, 512, -1)
        out[4 * i:4 * i + 4, 512 * f:512 * (f + 1), :][:, :, tq] = f32
    for f in (0, 1):
        t0, lo0 = k0s[f]
        if len(t0):
            out[:, 512 * f:512 * (f + 1), t0] = \
                feature[:, 512 * f:512 * (f + 1), lo0]
    return out


def kernel(feature, segments, max_len=T, **_unused):
    from concourse import bass_utils

    feature = np.asarray(feature, dtype=np.float32)
    assert feature.shape == (B, C2, T), feature.shape
    chunks, in_maps, perms, k0s = prepare(feature, segments, int(max_len))

    key = tuple(chunks)
    if _CACHE.get("key") != key:
        _CACHE["nc"] = _build_program(chunks)
        _CACHE["key"] = key
    nc = _CACHE["nc"]

    res = bass_utils.run_bass_kernel_spmd(
        nc, in_maps, core_ids=list(range(N_CORES)))
    return postprocess(res.results, perms, k0s, feature)


# revision 11
# speedup vs baseline: 2.9844x; 1.0135x over previous
"""BoundaryMaxPooling Trainium2 kernel, v7: half-time d=8 packed tables.

ap_gather cost is per-index and nearly flat in d (d=8 is only ~14%/idx
over d=4 while fetching 2x the data).  v7 therefore splits the TIME axis
in half (with a 257-wide halo, the max segment length) so SIXTEEN
half-tiles share one index stream, packed as 16 bf16 lanes per 32-byte
entry (d=8).  Each query belongs to exactly one half (by its lo), so the
per-core index count halves vs v4-v6.

Sharding: 8 cores = 2 families x 2 time-halves x 2 batch-groups.
Core c: family f=c//4 (start/end segments), half h=(c//2)%2 (data window
[767h, 767h+1281)), batch group i=c%2 (batches 4i..4i+3).  16 lanes =
4 batches x 4 channel-blocks of that family.

All 9 levels live in ONE tile pool of 4 rotating 41-KB buffers:
L0->b0, L1->b1, L2->b2, L3->b3, L4->b0, L5->b1, L6->b2, L7->b3, L8->b0.
Level k+4 overwrites level k's buffer, which class-k gathers read; since
gather SOURCE reads carry no completion semantics, each of L5..L8 is
preceded by a 2-column copy of class-k's last gather OUTPUT into the
target buffer -- the WAW edge with the build serializes it correctly
(with k<4 host-side no rotated-over buffer is ever read by a gather,
so no guard fires in practice).

Short queries (k<4: len<16, ~5%) and the rare k=8/k=0 queries are
computed on the host (~6% of output) -- their device cost is pure
per-instruction overhead crawling under the build chain.  Each chunk's
index stream is [all probe-1 | all probe-2]; the pairwise
max runs in place on the gather output.  Host packs bf16 input,
deinterleaves/upcasts/un-permutes the output.
"""

import numpy as np

B, C2, T = 8, 1024, 2048
P = 128
N_CORES = 8
KLEV = 9
HT = 1281                                # half window: 1024 + 257 halo
H_OFF = [0, 767]                         # window start per half
HN = [HT + 1 - (1 << k) for k in range(KLEV)]

MAXCH = 320                              # max queries per gather chunk
GROUPS = [4, 5, 6, 7]                   # k<4 and k=8 are host-side

_CACHE = {}


def _plan_chunks(counts):
    chunks = []
    for g in GROUPS:
        n = counts[g]
        while n > 0:
            c = min(n, MAXCH)
            chunks.append((g, ((c + 15) // 16) * 16))
            n -= c
    return chunks


def _build_program(chunks):
    import concourse.bacc as bacc
    import concourse.mybir as mybir
    import concourse.tile as tile

    bf16 = mybir.dt.bfloat16
    i16 = mybir.dt.int16
    i32 = mybir.dt.int32
    MAX = mybir.AluOpType.max

    qtot = sum(c for _, c in chunks)
    idxcols = 2 * qtot // 16

    nc = bacc.Bacc("TRN2", target_bir_lowering=False, debug=False,
                   num_devices=N_CORES)
    feat = nc.dram_tensor("feat", [P, 8 * HT], i32, kind="ExternalInput")
    idxw = nc.dram_tensor("idxw", [P, idxcols], i16, kind="ExternalInput")
    out = nc.dram_tensor("out", [P, 8 * qtot], i32, kind="ExternalOutput")

    with tile.TileContext(nc) as tc:
        with tc.tile_pool(name="tabp", bufs=4) as tp, \
             tc.tile_pool(name="gp", bufs=2) as gp, \
             tc.tile_pool(name="ip", bufs=1) as ip:
            idxt = ip.tile([P, idxcols], i16, tag="idx")
            nc.sync.dma_start(idxt[:], idxw[:])

            lev = [None] * KLEV
            lev[0] = tp.tile([P, 8 * HT], i32, tag="t", name="lev0")
            DB = [0, 320, 640, 960, HT]      # input DMA chunk boundaries
            for a, b in zip(DB, DB[1:]):
                nc.sync.dma_start(lev[0][:, 8 * a:8 * b],
                                  feat[:, 8 * a:8 * b])

            state = {"col": 0, "ocol": 0, "last": {}}

            def do_chunks(k):
                src = lev[k]
                for g, ch in chunks:
                    if g != k:
                        continue
                    ni = 2 * ch
                    gt = gp.tile([P, 16 * MAXCH], i32, tag="g")
                    nc.gpsimd.ap_gather(
                        gt[:, 0:8 * ni], src[:, 0:8 * HN[k]],
                        idxt[:, state["col"]:state["col"] + ni // 16],
                        channels=P, num_elems=HN[k], d=8, num_idxs=ni)
                    state["last"][k] = gt
                    gb = gt[:].bitcast(bf16)
                    nc.vector.tensor_tensor(
                        gb[:, 0:16 * ch],
                        gb[:, 0:16 * ch], gb[:, 16 * ch:32 * ch], MAX)
                    nc.sync.dma_start(
                        out[:, state["ocol"]:state["ocol"] + 8 * ch],
                        gt[:, 0:8 * ch])
                    state["col"] += ni // 16
                    state["ocol"] += 8 * ch

            for k in range(1, 8):
                lev[k] = tp.tile([P, 8 * HT], i32, tag="t",
                                 name=f"lev{k}")
                if k >= 5:
                    # guard: class k-4 gathers read the buffer this level
                    # overwrites; chain through their gather OUTPUT
                    ga = state["last"].get(k - 4)
                    if ga is not None:
                        nc.vector.tensor_copy(lev[k][:, 0:2], ga[:, 0:2])
                s = 1 << (k - 1)
                vo = lev[k][:].bitcast(bf16)
                vi = lev[k - 1][:].bitcast(bf16)
                # wavefront: each level builds in 4 pieces; piece j only
                # needs the previous level's pieces j and j+1 (range-level
                # dependency tracking), so levels pipeline diagonally
                QB = ([0, 318, 638, 958, HN[1]] if k == 1
                      else [0, 320, 640, 960, HN[k]])
                for a, b in zip(QB, QB[1:]):
                    nc.vector.tensor_tensor(
                        vo[:, 16 * a:16 * b],
                        vi[:, 16 * a:16 * b],
                        vi[:, 16 * (a + s):16 * (b + s)], MAX)
                do_chunks(k)
    nc.compile()
    return nc


def _f32_to_bf16_u16(x):
    u = x.astype(np.float32).view(np.uint32)
    rounded = u + 0x7FFF + ((u >> 16) & 1)
    return (rounded >> 16).astype(np.uint16)


def _queries(segments, max_len):
    seg = np.clip(np.asarray(segments, np.float32)[0], 0.0,
                  np.float32(max_len - 1))
    fams = []
    for f in (0, 1):
        lo = np.floor(seg[:, 2 * f]).astype(np.int64)
        hi = np.ceil(seg[:, 2 * f + 1]).astype(np.int64)
        hi = np.maximum(hi, lo + 1)
        ln = hi - lo
        k = np.floor(np.log2(ln.astype(np.float64))).astype(np.int64)
        fams.append((k, lo, hi - (1 << k)))
    return fams


def _layout(fams):
    """Chunk layout unified across the four (family, half) streams."""
    streams = []
    for f in (0, 1):
        k, p1, p2 = fams[f]
        for h in (0, 1):
            m = (k >= 4) & (k <= 7) & ((p1 >= 1024) == bool(h))
            streams.append((f, h, k, p1, p2, m))
    counts = {g: 0 for g in GROUPS}
    for _, _, k, _, _, m in streams:
        for g in GROUPS:
            counts[g] = max(counts[g], int(np.sum(m & (k == g))))
    chunks = _plan_chunks(counts)
    qtot = sum(c for _, c in chunks)

    lay = {}
    for f, h, k, p1, p2, m in streams:
        tsort = {g: np.nonzero(m & (k == g))[0] for g in GROUPS}
        used = {g: 0 for g in GROUPS}
        idx_stream, perm = [], []
        for g, ch in chunks:
            ts = tsort[g][used[g]:used[g] + ch]
            used[g] += ch
            npad = ch - len(ts)
            a = np.concatenate([p1[ts] - H_OFF[h],
                                np.zeros(npad, np.int64)])
            b = np.concatenate([p2[ts] - H_OFF[h],
                                np.zeros(npad, np.int64)])
            assert (a >= 0).all() and (b >= 0).all()
            assert (a < HN[g]).all() if len(ts) else True
            idx_stream.append(np.concatenate([a, b]))
            perm.append(np.concatenate([ts, -np.ones(npad, np.int64)]))
        idx = np.concatenate(idx_stream).astype(np.int16)
        assert idx.size == 2 * qtot
        wrapped = np.tile(idx.reshape(-1, 16).T, (8, 1)).astype(np.int16)
        lay[(f, h)] = (wrapped, np.concatenate(perm))
    k0s = []
    for f in (0, 1):
        k, p1, p2 = fams[f]
        th = np.nonzero((k < 4) | (k == 8))[0]
        # reconstruct hi = p2 + 2^k
        k0s.append((th, p1[th], p2[th] + (1 << k[th].astype(np.int64))))
    return chunks, lay, k0s


def prepare(feature, segments, max_len):
    feature = np.asarray(feature, np.float32)
    u16 = _f32_to_bf16_u16(feature)           # [B, C2, T]
    fams = _queries(segments, int(max_len))
    chunks, lay, k0s = _layout(fams)
    in_maps, perms = [], []
    for c in range(N_CORES):
        f, h, i = c // 4, (c // 2) % 2, c % 2
        # lanes j = 0..15: batch 4i + j//4, channels 512f + 128*(j%4) + p
        x = u16[4 * i:4 * i + 4, 512 * f:512 * (f + 1),
                H_OFF[h]:H_OFF[h] + HT]                    # [4,512,HT]
        x = x.reshape(4, 4, P, HT).transpose(2, 3, 0, 1)   # [p,e,b,cb]
        packed = np.ascontiguousarray(x.reshape(P, HT, 16)).view(np.uint32)
        packed = packed.reshape(P, 8 * HT).astype(np.int32, copy=False)
        wrapped, perm = lay[(f, h)]
        in_maps.append({"feat": packed, "idxw": wrapped})
        perms.append(perm)
    return chunks, in_maps, perms, k0s


def postprocess(results, perms, k0s, feature):
    feature = np.asarray(feature, np.float32)
    out = np.empty((B, C2, T), np.float32)
    for c in range(N_CORES):
        f, h, i = c // 4, (c // 2) % 2, c % 2
        r = np.asarray(results[c]["out"])          # [P, 8*qtot] i32
        qtot = r.shape[1] // 8
        u16 = r.view(np.uint16).reshape(P, qtot, 16)
        perm = perms[c]
        valid = perm >= 0
        tq = perm[valid]
        v = u16[:, valid, :]                       # [P, nq, 16]
        f32 = (v.astype(np.uint32) << 16).view(np.float32)
        f32 = f32.transpose(2, 0, 1).reshape(4, 4, P, -1).reshape(4, 512, -1)
        out[4 * i:4 * i + 4, 512 * f:512 * (f + 1), :][:, :, tq] = f32
    for f in (0, 1):
        th, loh, hih = k0s[f]
        ch = slice(512 * f, 512 * (f + 1))
        for t, lo, hi in zip(th, loh, hih):
            out[:, ch, t] = feature[:, ch, lo:hi].max(axis=-1)
    return out


def kernel(feature, segments, max_len=T, **_unused):
    from concourse import bass_utils

    feature = np.asarray(feature, dtype=np.float32)
    assert feature.shape == (B, C2, T), feature.shape
    chunks, in_maps, perms, k0s = prepare(feature, segments, int(max_len))

    key = tuple(chunks)
    if _CACHE.get("key") != key:
        _CACHE["nc"] = _build_program(chunks)
        _CACHE["key"] = key
    nc = _CACHE["nc"]

    res = bass_utils.run_bass_kernel_spmd(
        nc, in_maps, core_ids=list(range(N_CORES)))
    return postprocess(res.results, perms, k0s, feature)
